# revision 1
# baseline (speedup 1.0000x reference)
"""Trainium2 Bass kernel for nn_CrossNetwork (DCN-v1 cross network).

Math: reference computes x_{i+1} = input * (x_i . w_i) + x_i + b_i, L=6 layers.
Writing x_i = input * c_i + B_i with B_i = sum_{j<i} b_j (a constant row
vector) and c_i a per-row scalar, the recursion collapses to
    u_i    = input . w_i                     (per row, one tall-skinny matmul)
    beta_i = B_i . w_i                       (host-computed constants)
    c_{i+1} = c_i * (1 + u_i) + beta_i ; c_0 = 1
    out    = input * c_L + B_L
For the b == 0 case this is out = input * prod_i(1 + u_i).

Device work per core (2048 rows): load x once, PE-transpose 128x128 blocks,
fp32 matmul against W^T accumulating U[rows, 6], DVE product-reduce to c,
DVE per-partition-scalar multiply, store. ~2 passes over HBM => memory-bound.
"""

import numpy as np

import concourse.bass as bass
import concourse.mybir as mybir
import concourse.tile as tile
from concourse.bass_utils import run_bass_kernel_spmd
from concourse.masks import make_identity
from concourse.vector_clock import ScopedClock

F32 = mybir.dt.float32

B, D, L = 16384, 1024, 6
NCORES = 8
R = B // NCORES  # rows per core
P = 128
NCH = R // P  # chunks of 128 rows per core
KB = D // P  # 128-wide k blocks
GRP = 4  # chunks per psum accumulation group
DMA_SPLIT = False  # SP HWDGE alone measured faster (68 vs 76 us)
MM_F32R = False  # use float32r (full-rate) matmuls instead of fp32
NG = NCH // GRP


def _patch_tile_drain():
    """This walrus build rejects >1 sem wait on a CTRL (Drain) instruction.

    Tile's kernel-tail drain waits on every sem domain at once; split it into
    chained single-wait drains.
    """
    if getattr(tile.TileContext, "_drain_patched", False):
        return

    def _drain_and_barrier(self, tick_clock, wait_clock):
        gc = tick_clock.global_clock
        entries = [(proc, t) for proc, t in enumerate(gc) if t > 0]
        if not entries:
            self.nc.sync.drain()
        for proc, t in entries:
            sub = ScopedClock()
            sub.require_at_least(None, proc, t)
            drain_inst = self.nc.sync.drain()
            wait_clock.add_sem_waits(drain_inst.ins, sub)

        self.nc.all_engine_barrier()
        assert self.sems is not None
        popped = self.nc._tile_sem_poison_stack.pop()
        assert popped is self._sem_poison
        self.nc.clear_and_free_semaphores(list(self.sems.allocated().values()))

    tile.TileContext._drain_and_barrier = _drain_and_barrier
    tile.TileContext._drain_patched = True


def _build(with_bias: bool, loop_n: int = 1, mode: str = "full"):
    nc = bass.Bass("TRN2")
    x_d = nc.dram_tensor("x", [R, D], F32, kind="ExternalInput")
    wt_d = nc.dram_tensor("wt", [D, L], F32, kind="ExternalInput")
    if with_bias:
        bl_d = nc.dram_tensor("bl", [1, D], F32, kind="ExternalInput")
        beta_d = nc.dram_tensor("beta", [1, L], F32, kind="ExternalInput")
    y_d = nc.dram_tensor("y", [R, D], F32, kind="ExternalOutput")

    xv = x_d.rearrange("(p n) d -> p n d", p=P)  # [128, NCH, D]
    yv = y_d.rearrange("(p n) d -> p n d", p=P)
    wtv = wt_d.rearrange("(k p) s -> p k s", p=P)  # [128, KB, L]

    with tile.TileContext(nc) as tc:
        with (
            tc.tile_pool(name="consts", bufs=1) as consts,
            tc.tile_pool(name="xch", bufs=NCH) as xpool,
            tc.tile_pool(name="xt", bufs=6) as xtpool,
            tc.tile_pool(name="small", bufs=2 * GRP) as small,
            tc.tile_pool(name="pxt", bufs=4, space="PSUM") as pxt,
            tc.tile_pool(name="pu", bufs=2, space="PSUM") as pu,
        ):
            ident = consts.tile([P, P], F32)
            make_identity(nc, ident)
            ident6 = consts.tile([L, L], F32)
            make_identity(nc, ident6)
            wt_sb = consts.tile([P, KB, L], F32)
            nc.sync.dma_start(out=wt_sb, in_=wtv)
            if with_bias:
                bl_sb = consts.tile([P, D], F32)
                nc.sync.dma_start(
                    out=bl_sb,
                    in_=bass.AP(tensor=bl_d, offset=0, ap=[[0, P], [1, D]]),
                )
                beta_sb = consts.tile([P, L], F32)
                nc.sync.dma_start(
                    out=beta_sb,
                    in_=bass.AP(tensor=beta_d, offset=0, ap=[[0, P], [1, L]]),
                )

            import contextlib
            loop_cm = (
                tc.For_i(0, loop_n, 1) if loop_n > 1 else contextlib.nullcontext()
            )
            if mode == "compute":
                x_pre = []
                for n in range(NCH):
                    xt_pre = xpool.tile([P, D], F32, tag="xch", name=f"xpre{n}")
                    nc.sync.dma_start(out=xt_pre, in_=xv[:, n, :])
                    x_pre.append(xt_pre)
            else:
                x_pre = None
            with loop_cm:
                _body(nc, tc, xpool, xtpool, small, pxt, pu, consts, ident,
                      ident6, wt_sb, locals().get("bl_sb"),
                      locals().get("beta_sb"), xv, yv, with_bias, mode, x_pre)
    return nc


def _body(nc, tc, xpool, xtpool, small, pxt, pu, consts, ident, ident6,
          wt_sb, bl_sb, beta_sb, xv, yv, with_bias, mode="full", x_pre=None):
            if mode == "compute":
                x_sb = x_pre
            else:
                x_sb = []
                for n in range(NCH):
                    xt_tile = xpool.tile([P, D], F32, tag="xch")
                    eng = nc.sync if (not DMA_SPLIT or n % 2 == 0) else nc.scalar
                    eng.dma_start(out=xt_tile, in_=xv[:, n, :])
                    x_sb.append(xt_tile)
            if mode == "dma":
                for n in range(NCH):
                    eng = nc.scalar if (DMA_SPLIT and n % 2 == 0) else nc.sync
                    eng.dma_start(out=yv[:, n, :], in_=x_sb[n])
                return

            for g in range(NG):
                # U^T[6, 512] accumulated over k blocks; stationary weights
                # are only 6 columns so LDWEIGHTS is trivial.
                ut_ps = pu.tile([L, GRP * P], F32, tag="u", name=f"ut{g}")
                for k in range(KB):
                    pxt_t = pxt.tile([P, GRP * P], F32, tag="pxt")
                    for j in range(GRP):
                        nc.tensor.transpose(
                            pxt_t[:, j * P : (j + 1) * P],
                            x_sb[g * GRP + j][:, k * P : (k + 1) * P],
                            ident,
                        )
                    xt_t = xtpool.tile([P, GRP * P], F32, tag="xt")
                    nc.scalar.copy(xt_t, pxt_t)
                    lhs = wt_sb[:, k, :]
                    rhs = xt_t[:]
                    if MM_F32R:
                        lhs = lhs.bitcast(mybir.dt.float32r)
                        rhs = rhs.bitcast(mybir.dt.float32r)
                    nc.tensor.matmul(
                        ut_ps[:],
                        lhs,
                        rhs,
                        start=(k == 0),
                        stop=(k == KB - 1),
                    )
                # 1 + U^T on ACT while copying PSUM->SBUF, then transpose
                # [6,128] blocks back to row-major [128,6] per chunk.
                u1t_t = xtpool.tile([L, GRP * P], F32, tag="u1t")
                nc.vector.tensor_scalar_add(u1t_t, ut_ps, 1.0)
                uj_ps = pu.tile([P, GRP, L], F32, tag="uj", name=f"uj{g}")
                for j in range(GRP):
                    nc.tensor.transpose(
                        uj_ps[:, j, :],
                        u1t_t[:, j * P : (j + 1) * P],
                        ident6,
                    )
                for j in range(GRP):
                    n = g * GRP + j
                    u1_t = small.tile([P, L], F32, tag="u1")
                    nc.vector.tensor_copy(u1_t, uj_ps[:, j, :])
                    if not with_bias:
                        # c = prod over the 6 (1+u_i): 3 pairwise muls
                        p3_t = small.tile([P, 3], F32, tag="p3")
                        nc.vector.tensor_mul(p3_t, u1_t[:, 0:3], u1_t[:, 3:6])
                        p1_t = small.tile([P, 1], F32, tag="p1")
                        nc.vector.tensor_mul(p1_t, p3_t[:, 0:1], p3_t[:, 1:2])
                        c_t = small.tile([P, 1], F32, tag="c")
                        nc.vector.tensor_mul(c_t, p1_t, p3_t[:, 2:3])
                        if mode == "compute":
                            scr_t = xtpool.tile([P, D], F32, tag="scr")
                            nc.vector.tensor_scalar_mul(scr_t, x_sb[n], c_t)
                        else:
                            nc.vector.tensor_scalar_mul(x_sb[n], x_sb[n], c_t)
                    else:
                        c_t = small.tile([P, 1], F32, tag="c")
                        nc.vector.memset(c_t, 1.0)
                        for i in range(L):
                            # c = c * (1 + u_i) + beta_i
                            nc.vector.scalar_tensor_tensor(
                                out=c_t,
                                in0=c_t,
                                scalar=u1_t[:, i : i + 1],
                                in1=beta_sb[:, i : i + 1],
                                op0=mybir.AluOpType.mult,
                                op1=mybir.AluOpType.add,
                            )
                        # out = x * c + B_L
                        nc.vector.scalar_tensor_tensor(
                            out=x_sb[n],
                            in0=x_sb[n],
                            scalar=c_t,
                            in1=bl_sb,
                            op0=mybir.AluOpType.mult,
                            op1=mybir.AluOpType.add,
                        )
                    if mode != "compute":
                        eng = nc.scalar if (DMA_SPLIT and n % 2 == 0) else nc.sync
                        eng.dma_start(out=yv[:, n, :], in_=x_sb[n])


def _split_multi_waits(nc):
    """This walrus build allows only one sem wait on several instruction
    structs (e.g. self-loading Matmult). Move extra waits onto preceding
    same-engine NOPs; engine FIFO order makes this equivalent."""
    n = 0
    for fn in nc.m.functions:
        for bb in fn.blocks:
            out = []
            for inst in bb.instructions:
                si = inst.sync_info
                if si is not None and si.on_wait and len(si.on_wait) > 1:
                    waits = list(si.on_wait)
                    for w in waits[:-1]:
                        n += 1
                        out.append(
                            mybir.InstNoOp(
                                name=f"nopw-{n}-{inst.name}",
                                engine=inst.engine,
                                sync_info=mybir.SyncInfo(
                                    on_wait=[w], on_update=[]
                                ),
                                bass_nofuse=True,
                            )
                        )
                    inst.sync_info = mybir.SyncInfo(
                        on_wait=[waits[-1]], on_update=list(si.on_update)
                    )
                out.append(inst)
            bb.instructions = out


_CACHE = {}


def _get_nc(with_bias: bool, loop_n: int = 1, mode: str = "full"):
    key = (with_bias, loop_n, mode, DMA_SPLIT, MM_F32R)
    if key not in _CACHE:
        _patch_tile_drain()
        nc = _build(with_bias, loop_n, mode)
        _split_multi_waits(nc)
        _CACHE[key] = nc
    return _CACHE[key]


def kernel(input, W, b, **run_kwargs):
    input = np.ascontiguousarray(np.asarray(input, dtype=np.float32))
    W = np.asarray(W, dtype=np.float32)
    b = np.asarray(b, dtype=np.float32)
    assert input.shape == (B, D) and W.shape == (L, D) and b.shape == (L, D)

    with_bias = bool(np.any(b))
    nc = _get_nc(with_bias)

    wt = np.ascontiguousarray(W.T)  # [D, L]
    in_maps = []
    for i in range(NCORES):
        m = {"x": input[i * R : (i + 1) * R], "wt": wt}
        if with_bias:
            # B_i = sum_{j<i} b_j ; beta_i = B_i . w_i ; B_L = sum_j b_j
            Bpre = np.concatenate(
                [np.zeros((1, D), np.float32), np.cumsum(b, axis=0)[:-1]], axis=0
            )
            m["bl"] = b.sum(axis=0, dtype=np.float32).reshape(1, D)
            m["beta"] = np.einsum("ld,ld->l", Bpre, W).astype(np.float32).reshape(1, L)
        in_maps.append(m)

    res = run_bass_kernel_spmd(
        nc, in_maps, core_ids=list(range(NCORES)), **run_kwargs
    )
    out = np.concatenate([res.results[i]["y"] for i in range(NCORES)], axis=0)
    if run_kwargs:
        return out, res
    return out

